# revision 1
# baseline (speedup 1.0000x reference)
"""Trainium2 Bass kernel for BinaryLinearWscales.

Math:  out = x @ (wscale * sign(weight) + wbias).T
     = wscale_n * (x @ sign(weight).T)_tn + wbias_n * rowsum(x)_t

Key trick: the matmul runs in **float32r** (fp32 stored, rounded by HW to
8-bit exp + 11-bit mantissa), which streams at 1 cycle/row on the PE for
N>=256 — bf16 speed at ~2.4e-4 per-element rounding.  sign(weight) = +-1
is EXACT in f32r, and only x is rounded, so end-to-end relative error is
~1e-4 (verified on HW).  Fallback modes: 'bf16x2' (hi/lo split two-pass,
~2.5e-5 err, 2x PE time) and 'bf16' (one-pass, ~2e-3 err).

Sharding (tensor-parallel over DOUT): each of the 8 cores gets 512 rows
of weight/wscale/wbias and the full x; host concatenates core outputs
along the feature dim.

Per-core pipeline (all fully unrolled under the Tile framework):
  - weights: SWDGE DMA with f32->bf16 cast in flight (sign-invariant) ->
    in-place sign on DVE ((w>=0)*2-1) -> PE transpose (bf16, via identity)
    -> PSUM -> ACT copyback to SBUF as f32r (swT[kc] [128k x 512n] tiles,
    cached all kernel; kc-ordered so completion matches matmul consumption)
  - x: SWDGE DMA natural [t,k] (1MB halves; first tile in 512KB quarters,
    issued before all other gpsimd work so the PE starts earliest) with
    f32 -> f32r cast in
    flight (walrus accepts the cast-DMA as the f32r-rounding producer;
    HW-verified bit-identical to engine-side rounding) -> PE transpose
    (f32r, 1.5 cyc/row, via identity) -> PSUM -> plain copyback,
    alternating ACT/DVE
  - matmuls: psum_out[t128, n512] += xT.T @ swT over 32 k-chunks;
    psum_xsum[t128, 8] += xT.T @ ones8 (rowsum of x for the wbias term;
    f32r rejects N=1, hence the 8-wide ones)
  - global software pipeline over chunks g = tg*32 + kc: transposes run
    PRESTAGE=12 chunks ahead of the matmuls, crossing token-group
    boundaries, so PSUM copybacks hide under matmul streams
  - epilogue (DVE): out = psum_out * wscale_rep + wbias_rep * xsum via
    tensor_mul + scalar_tensor_tensor (wscale/wbias replicated across
    partitions once via gpsimd partition_broadcast)

Measured (8 cores, axon): rel err 1.0298e-04 (stable across all HW
runs); TimelineSim predicts 366.5 us at 87.7% PE occupancy (221 us f32r
matmul streams + 84 us f32r transposes + 14 us xsum + ~7 us weight
phase + ~27 us startup/tail). An earlier variant's clean paired
wall-clock measurement matched its model within 4%.
"""

import os
from contextlib import ExitStack

import numpy as np

P = 128

# full problem dims
B, S, DIN, DOUT = 2, 2048, 4096, 4096
N_CORES = 8
N_SHARD = DOUT // N_CORES  # 512


def build_body(ctx, tc, out_ap, x_ap, w_ap, wscale_ap, wbias_ap, mode="f32r"):
    """mode: 'f32r' (single-pass fp32r matmul, ~1e-4 err),
    'bf16x2' (hi/lo two-pass bf16, ~2.5e-5 err),
    'bf16' (single-pass bf16, ~2e-3 err)."""
    import concourse.bass as bass
    from concourse import mybir
    from concourse.bass import ts
    from concourse.masks import make_identity

    nc = tc.nc
    T, K = x_ap.shape
    N, K2 = w_ap.shape
    assert K == K2
    assert T % 256 == 0 and K % P == 0 and N % P == 0 and N <= 512
    KC = K // P  # k chunks
    NB = N // P  # weight row blocks
    TGRP = 256  # tokens per transpose group
    TB = TGRP // P  # 2
    NTG = T // TGRP
    two_pass = mode == "bf16x2"

    f32 = mybir.dt.float32
    bf16 = mybir.dt.bfloat16
    f32r = mybir.dt.float32r
    u32 = mybir.dt.uint32
    Alu = mybir.AluOpType
    # dtype used for matmul operands
    mm_dt = f32r if mode == "f32r" else bf16

    # ---------------- pools (SBUF; PSUM pools open after weight phase) ----
    xnat_pool = ctx.enter_context(tc.tile_pool(name="xnat", bufs=4))
    xsplit_pool = ctx.enter_context(tc.tile_pool(name="xsplit", bufs=16))
    opool = ctx.enter_context(tc.tile_pool(name="opool", bufs=3))

    def load_x_group(tg):
        """DMA the tg-th 256-token block of x.

        In f32r mode the SWDGE DMA casts f32 -> f32r in flight, so the PE
        transposes run at 1.5 cyc/row instead of f32's 2.0 and no separate
        rounding pass is needed."""
        x_nats = []
        for tb in range(TB):
            x_nat = xnat_pool.tile(
                [P, K], mm_dt if mode == "f32r" else f32,
                name=f"x_nat_{tg}_{tb}", tag="x_nat", bufs=4,
            )
            row = ts(tg * TB + tb, P)
            dma = nc.gpsimd.dma_start if mode == "f32r" else nc.sync.dma_start
            pieces = 4 if (tg == 0 and tb == 0) else 2
            step = K // pieces
            for pc in range(pieces):
                dma(x_nat[:, pc * step:(pc + 1) * step],
                    x_ap[row, pc * step:(pc + 1) * step])
            x_nats.append(x_nat)
        return x_nats

    # prefetch the first x group before the weight phase so the DVE/PE can
    # start on x while weights stream in
    x_groups = {0: load_x_group(0)}

    # ---------------- constants ----------------
    const = ctx.enter_context(tc.tile_pool(name="const", bufs=1))
    ident_f32 = const.tile([P, P], f32, name="ident_f32", tag="ident_f32")
    make_identity(nc, ident_f32)
    ident_bf16 = const.tile([P, P], bf16, name="ident_bf16", tag="ident_bf16")
    nc.vector.tensor_copy(ident_bf16[:], ident_f32[:])
    ident_mm = const.tile([P, P], mm_dt, name="ident_mm", tag="ident_mm")
    nc.vector.tensor_copy(ident_mm[:], ident_f32[:])
    # fp32r matmuls reject free-dim 1; use an 8-wide ones block for xsum
    ONESW = 8
    ones_stage = const.tile([P, ONESW], f32, name="ones_stage", tag="ones_stage")
    nc.vector.memset(ones_stage[:], 1.0)
    ones_col = const.tile([P, ONESW], mm_dt, name="ones_col", tag="ones_col")
    nc.vector.tensor_copy(ones_col[:], ones_stage[:])

    # wscale / wbias replicated across all 128 partitions
    wsc_stage = const.tile([1, N], f32, name="wsc_stage", tag="wsc_stage")
    nc.sync.dma_start(wsc_stage[:], wscale_ap[:, :])
    wbi_stage = const.tile([1, N], f32, name="wbi_stage", tag="wbi_stage")
    nc.sync.dma_start(wbi_stage[:], wbias_ap[:, :])
    wscale_rep = const.tile([P, N], f32, name="wscale_rep", tag="wscale_rep")
    nc.gpsimd.partition_broadcast(wscale_rep[:], wsc_stage[:])
    wbias_rep = const.tile([P, N], f32, name="wbias_rep", tag="wbias_rep")
    nc.gpsimd.partition_broadcast(wbias_rep[:], wbi_stage[:])

    def get_x(tg):
        if tg not in x_groups:
            x_groups[tg] = load_x_group(tg)
        return x_groups[tg]

    psx_pool = ctx.enter_context(tc.tile_pool(name="psx", bufs=3, space="PSUM"))

    def stage_chunk(g):
        """PE-transpose global chunk g (= tg*KC + kc) and copy it back."""
        tg, kc = divmod(g, KC)
        x_nats = get_x(tg)
        psx = psx_pool.tile(
            [P, TGRP], mm_dt if mode == "f32r" else f32,
            name=f"psx_{tg}_{kc}", tag="psx",
        )
        for tb in range(TB):
            nc.tensor.transpose(
                psx[:, ts(tb, P)],
                x_nats[tb][:, ts(kc, P)],
                ident_mm if mode == "f32r" else ident_f32,
            )
        # copyback PSUM -> SBUF with cast (f32r rounding / bf16 cast happens
        # here); alternate ACT/DVE to balance engine load
        xhi = xsplit_pool.tile([P, TGRP], mm_dt, name=f"xhi_{tg}_{kc}", tag="xhi")
        if kc % 2 == 0:
            nc.scalar.copy(xhi[:], psx[:])
        else:
            nc.vector.tensor_copy(xhi[:], psx[:])
        xlo = None
        if two_pass:
            xlo = xsplit_pool.tile([P, TGRP], bf16, name=f"xlo_{tg}_{kc}", tag="xlo")
            nc.vector.tensor_sub(xlo[:], psx[:], xhi[:])
        return xhi, xlo

    # pre-stage early chunks: fills the PE while the weight phase waits on
    # the weight DMA + sign
    NCHUNK = NTG * KC
    PRESTAGE = 12
    staged = [stage_chunk(g) for g in range(min(PRESTAGE, NCHUNK))]

    # ---------------- weight phase ----------------
    # sw_T[kc]: [128 k, N n] tiles of sign(w).T, cached for whole kernel
    swt_pool = ctx.enter_context(tc.tile_pool(name="swt", bufs=1))
    swT = [
        swt_pool.tile([P, N], mm_dt, name=f"swT{kc}", tag=f"swT{kc}")
        for kc in range(KC)
    ]
    with tc.tile_pool(name="wphase", bufs=1) as wpool, tc.tile_pool(
        name="wpsum", bufs=2, space="PSUM"
    ) as wpsum_pool:
        s_nats = []
        for nb in range(NB):
            # SWDGE DMA with f32->bf16 cast in flight (halves the weight
            # traffic; sign is invariant to bf16 rounding)
            s_nat = wpool.tile([P, K], bf16, name=f"w_nat{nb}", tag="w_nat", bufs=4)
            nc.gpsimd.dma_start(s_nat[:], w_ap[ts(nb, P), :])
            # sign in place on DVE: s = (w >= 0) * 2 - 1  -> exactly +-1.0
            nc.vector.tensor_scalar(
                out=s_nat[:],
                in0=s_nat[:],
                scalar1=0.0,
                scalar2=2.0,
                op0=Alu.is_ge,
                op1=Alu.mult,
            )
            nc.vector.tensor_scalar(
                out=s_nat[:],
                in0=s_nat[:],
                scalar1=1.0,
                scalar2=None,
                op0=Alu.subtract,
            )
            s_nats.append(s_nat)
        # kc-outer so swT[kc] completes in the order the matmuls consume it
        for kc in range(KC):
            pw = wpsum_pool.tile([P, N], bf16, name=f"pw_{kc}", tag="pw")
            for nb in range(NB):
                nc.tensor.transpose(
                    pw[:, ts(nb, P)], s_nats[nb][:, ts(kc, P)], ident_bf16
                )
            nc.scalar.copy(swT[kc][:], pw[:])  # bf16 -> mm_dt, exact for +-1

    # ---------------- main phase ----------------
    # global software pipeline over chunks g = tg*KC + kc: transposes run
    # PRESTAGE chunks ahead of the matmuls, crossing token-group boundaries.
    pox_pool = ctx.enter_context(tc.tile_pool(name="pox", bufs=3, space="PSUM"))
    pss_pool = ctx.enter_context(tc.tile_pool(name="pss", bufs=2, space="PSUM"))

    for tg in range(NTG):
        psum_os = [
            pox_pool.tile([P, 512], f32, name=f"po_{tg}_{ot}", tag="po")[:, :N]
            for ot in range(TB)
        ]
        psum_ss = [
            pss_pool.tile([P, 8], f32, name=f"ps_{tg}_{ot}", tag="ps")
            for ot in range(TB)
        ]
        for kc in range(KC):
            xhi, xlo = staged.pop(0)
            g = tg * KC + kc
            if g + PRESTAGE < NCHUNK:
                staged.append(stage_chunk(g + PRESTAGE))
            for ot in range(TB):
                lhs_hi = xhi[:, ts(ot, P)]
                nc.tensor.matmul(
                    psum_os[ot],
                    lhs_hi,
                    swT[kc][:],
                    start=(kc == 0),
                    stop=(not two_pass and kc == KC - 1),
                )
                nc.tensor.matmul(
                    psum_ss[ot][:, 0:ONESW],
                    lhs_hi,
                    ones_col[:],
                    start=(kc == 0),
                    stop=(kc == KC - 1),
                )
                if two_pass:
                    lhs_lo = xlo[:, ts(ot, P)]
                    nc.tensor.matmul(
                        psum_os[ot],
                        lhs_lo,
                        swT[kc][:],
                        start=False,
                        stop=(kc == KC - 1),
                    )

        for ot in range(TB):
            psum_o = psum_os[ot]
            psum_s = psum_ss[ot]
            out_sb = opool.tile([P, N], f32, name=f"out_sb_{tg}_{ot}", tag="out_sb")
            nc.vector.tensor_mul(out_sb[:], psum_o, wscale_rep[:])
            nc.vector.scalar_tensor_tensor(
                out=out_sb[:],
                in0=wbias_rep[:],
                scalar=psum_s[:, 0:1],
                in1=out_sb[:],
                op0=Alu.mult,
                op1=Alu.add,
            )
            nc.sync.dma_start(out_ap[ts(tg * TB + ot, P), :], out_sb[:])


def build_nc(T, K, N, mode="f32r"):
    import concourse.tile as tile
    from concourse import bacc, mybir

    nc = bacc.Bacc(
        "TRN2",
        target_bir_lowering=False,
        debug=False,
        enable_asserts=False,
    )
    f32 = mybir.dt.float32
    x_t = nc.dram_tensor("x", [T, K], f32, kind="ExternalInput")
    w_t = nc.dram_tensor("w", [N, K], f32, kind="ExternalInput")
    wsc_t = nc.dram_tensor("wscale", [1, N], f32, kind="ExternalInput")
    wbi_t = nc.dram_tensor("wbias", [1, N], f32, kind="ExternalInput")
    out_t = nc.dram_tensor("out", [T, N], f32, kind="ExternalOutput")

    with tile.TileContext(nc) as tc:
        with ExitStack() as ctx:
            build_body(
                ctx,
                tc,
                out_t.ap(),
                x_t.ap(),
                w_t.ap(),
                wsc_t.ap(),
                wbi_t.ap(),
                mode=mode,
            )
    nc.compile()
    return nc


_NC_CACHE = {}
_LAST_RESULT = None


def _get_nc(T, K, N, mode):
    key = (T, K, N, mode)
    if key not in _NC_CACHE:
        _NC_CACHE[key] = build_nc(T, K, N, mode)
    return _NC_CACHE[key]


def _make_in_maps(inputs):
    x = inputs["x"] if "x" in inputs else inputs.get("x")
    weight = inputs["weight"]
    wscale = inputs["wscale"]
    wbias = inputs["wbias"]
    x = np.ascontiguousarray(np.asarray(x, dtype=np.float32).reshape(B * S, DIN))
    weight = np.asarray(weight, dtype=np.float32)
    wscale = np.asarray(wscale, dtype=np.float32).reshape(-1)
    wbias = np.asarray(wbias, dtype=np.float32).reshape(-1)
    in_maps = []
    for c in range(N_CORES):
        sl = slice(c * N_SHARD, (c + 1) * N_SHARD)
        in_maps.append(
            {
                "x": x,
                "w": np.ascontiguousarray(weight[sl]),
                "wscale": np.ascontiguousarray(wscale[sl]).reshape(1, N_SHARD),
                "wbias": np.ascontiguousarray(wbias[sl]).reshape(1, N_SHARD),
            }
        )
    return in_maps


def kernel(x, weight, wscale, wbias):
    from concourse.bass_utils import run_bass_kernel_spmd

    mode = os.environ.get("KERNEL_MODE", "f32r")
    nc = _get_nc(B * S, DIN, N_SHARD, mode)
    in_maps = _make_in_maps(
        {"x": x, "weight": weight, "wscale": wscale, "wbias": wbias}
    )

    trace = os.environ.get("KERNEL_TRACE", "0") == "1"
    res = run_bass_kernel_spmd(
        nc, in_maps, core_ids=list(range(N_CORES)), trace=trace
    )
    global _LAST_RESULT
    _LAST_RESULT = res
    if trace and res.exec_time_ns is not None:
        print(f"HW exec time: {res.exec_time_ns} ns")
    outs = [res.results[c]["out"] for c in range(N_CORES)]
    full = np.concatenate(outs, axis=1)  # [T, DOUT]
    return full.reshape(B, S, DOUT).astype(np.float32)

